# revision 26
# baseline (speedup 1.0000x reference)
"""CLUB loss kernel for 8x TRN2 NeuronCores.

Math: per sample b (L=512 positions, D=64 dims):
  mu     = MLP_mu(x);  logvar = tanh(MLP_lv(x));  iv = exp(-logvar)
  loss = -0.5/(B*L) * sum_{b,d,l} [ ((ysq - Ey2) - mu*yd2) * iv ]
with ysq = y^2, yd2 = 2*(y - Ey); Ey/Ey2 per-(b,d) means over l.

y never feeds a matmul, so ysq/yd2/Ey2 are precomputed host-side and
shipped (bf16) instead of y.

Layer 1 runs in fp8 e4m3 DoubleRow mode (2 MACs/PE-row/cycle): the
192-channel contraction packs as 96 partitions x 2 rows, so one
matmul per (path, L-half) replaces the bf16 a/b split pair and the PE
spine halves. w1 ships x8 (lifting ~N(0,0.05) weights out of the e4m3
subnormal range); relu is positive-homogeneous so hs = relu(8h + 8*b1)
and the 8x cancels via w2/8 shipped host-side. Quantization errors are
random-sign across 32K summed terms; measured end-to-end error stays
~1e-3.

Everything after layer 1 runs in a (d, L-half) stacked layout -
partition p<64 is (d=p, half 0), p>=64 is (d=p-64, half 1) - so
tanh/exp/m2/v/fin are single full-width [128, 256] ops. Ey2 is folded
into the final DVE op: fin = ((ysq - m2) - Ey2)*iv accumulated
per-partition, so the scalar loss needs only one ones-vector collapse
matmul and no ACT accumulator on the critical path.

DMA: two packed [128, W] bf16-typed inputs (fp8/f32 regions ride in
them and are bitcast on-chip), triggered on SP and ACT HWDGE, 128
descriptors each. Output store is [4, 1] f32.

Sharding: data-parallel over batch B=8, one sample per core; host
does the tiny final combine.
"""

import sys

if "/opt/trn_rl_repo" not in sys.path:
    sys.path.insert(0, "/opt/trn_rl_repo")

import numpy as np

B, L = 8, 512
XD, YD, H = 192, 64, 128
NCORES = 8
HC = L // 2

WA = 768   # bf16-cols: w1lv8 128 | w1mu8 128 | x8 half0 256 | x8 half1 256
WB = 652   # xa-free: w2lv 64 | w2mu 64 | ysq2 256 | yd22 256 | consts 12

_CACHE: dict = {}


def build_nc(debug: bool = False):
    import concourse.bass as bass
    import concourse.bacc as bacc
    import concourse.tile as tile
    from concourse import mybir
    from concourse.tile import add_dep_helper

    f32 = mybir.dt.float32
    bf16 = mybir.dt.bfloat16
    f8 = mybir.dt.float8e4
    AF = mybir.ActivationFunctionType
    OP = mybir.AluOpType
    DR = mybir.MatmulPerfMode.DoubleRow

    nc = bacc.Bacc("TRN2", target_bir_lowering=False, debug=debug)

    pa_d = nc.dram_tensor("pa", [96, 512], bf16, kind="ExternalInput")
    px_d = nc.dram_tensor("px", [96, 256], bf16, kind="ExternalInput")
    pb_d = nc.dram_tensor("pb", [128, WB], bf16, kind="ExternalInput")
    acc_d = nc.dram_tensor("acc", [4, 1], f32, kind="ExternalOutput")

    with tile.TileContext(nc) as tc:
        with (
            tc.tile_pool(name="sb", bufs=1) as sb,
            tc.tile_pool(name="ps", bufs=1, space=bass.MemorySpace.PSUM) as ps,
        ):
            pa = sb.tile([96, 512], bf16, tag="pa")
            nc.sync.dma_start(out=pa, in_=pa_d[:, :])
            px = sb.tile([96, 256], bf16, tag="px")
            mm_px = nc.scalar.dma_start(out=px, in_=px_d[:, :])
            pb = sb.tile([128, WB], bf16, tag="pb")
            mm_pb = nc.scalar.dma_start(out=pb, in_=pb_d[:, :])
            add_dep_helper(mm_pb.ins, mm_px.ins, sync=False, reason="act-q-order")

            def dr3(ap, m):  # [96, 2m fp8] -> [96, 2, m] DoubleRow operand
                return ap.bitcast(f8).rearrange("p (two f) -> p two f", two=2)

            w1lv8 = dr3(pa[0:96, 0:128], 128)     # [96, 2, 128]
            w1mu8 = dr3(pa[0:96, 128:256], 128)
            x8 = [dr3(pa[0:96, 256:512], 256), dr3(px[0:96, 0:256], 256)]
            w2lvT = pb[:, 0:64]    # w2/8, bf16
            w2muT = pb[:, 64:128]
            ysq2 = pb[:, 128:384]      # (d, half) stacked
            yd22 = pb[:, 384:640]
            b1lv8 = pb[:, 640:642].bitcast(f32)   # 8*b1
            b1mu8 = pb[:, 642:644].bitcast(f32)
            b2lv = pb[:, 644:646].bitcast(f32)    # rows duplicated per half
            b2mu = pb[:, 646:648].bitcast(f32)
            ey2c = pb[:, 648:650].bitcast(f32)    # Ey2 dup
            ones = pb[:, 650:652].bitcast(f32)

            acct = sb.tile([128, 4], f32, tag="acct")
            nc.gpsimd.memset(acct, 0.0)

            hs_lv = sb.tile([128, L], bf16, tag="hslv")
            hs_mu = sb.tile([128, L], bf16, tag="hsmu")
            tt = sb.tile([128, HC], f32, tag="tt")
            ivd = sb.tile([128, HC], bf16, tag="ivd")

            # layer 1: fp8 DoubleRow, one matmul per (path, half)
            h_lv0 = ps.tile([128, HC], f32, tag="hlv0")
            h_lv1 = ps.tile([128, HC], f32, tag="hlv1")
            h_mu0 = ps.tile([128, HC], f32, tag="hmu0")
            h_mu1 = ps.tile([128, HC], f32, tag="hmu1")
            dlv0 = nc.tensor.matmul(h_lv0, w1lv8, x8[0], start=True, stop=True,
                                    perf_mode=DR)
            dlv1 = nc.tensor.matmul(h_lv1, w1lv8, x8[1], start=True, stop=True,
                                    perf_mode=DR)
            dmu0 = nc.tensor.matmul(h_mu0, w1mu8, x8[0], start=True, stop=True,
                                    perf_mode=DR)
            dmu1 = nc.tensor.matmul(h_mu1, w1mu8, x8[1], start=True, stop=True,
                                    perf_mode=DR)

            # relus emit 8*relu(h + b1); the 8x cancels in w2/8.
            # relu_mu1 runs on DVE to keep the ACT spine short.
            r_lv0 = nc.scalar.activation(
                out=hs_lv[:, 0:HC], in_=h_lv0, func=AF.Relu, bias=b1lv8, scale=1.0
            )
            r_lv1 = nc.scalar.activation(
                out=hs_lv[:, HC:L], in_=h_lv1, func=AF.Relu, bias=b1lv8, scale=1.0
            )
            r_mu0 = nc.scalar.activation(
                out=hs_mu[:, 0:HC], in_=h_mu0, func=AF.Relu, bias=b1mu8, scale=1.0
            )
            r_mu1 = nc.vector.tensor_scalar(
                out=hs_mu[:, HC:L], in0=h_mu1, scalar1=b1mu8, scalar2=0.0,
                op0=OP.add, op1=OP.max,
            )

            # layer 2 (bf16) into (d, half) stacked PSUM tiles
            nbLV = ps.tile([128, HC], f32, tag="nblv")
            nbMU = ps.tile([128, HC], f32, tag="nbmu")
            w2lv0 = nc.tensor.matmul(
                nbLV[0:64, :], w2lvT, hs_lv[:, 0:HC], start=True, stop=True
            )
            w2lv1 = nc.tensor.matmul(
                nbLV[64:128, :], w2lvT, hs_lv[:, HC:L], start=True, stop=True
            )
            w2mu0 = nc.tensor.matmul(
                nbMU[0:64, :], w2muT, hs_mu[:, 0:HC], start=True, stop=True
            )
            w2mu1 = nc.tensor.matmul(
                nbMU[64:128, :], w2muT, hs_mu[:, HC:L], start=True, stop=True
            )

            # lv tail: tanh(+b2lv) -> exp(-.)
            a_tanh = nc.scalar.activation(
                out=tt, in_=nbLV, func=AF.Tanh, bias=b2lv, scale=1.0
            )
            a_exp_a = nc.scalar.activation(
                out=ivd[:, 0:128], in_=tt[:, 0:128], func=AF.Exp, scale=-1.0
            )
            a_exp_b = nc.scalar.activation(
                out=ivd[:, 128:256], in_=tt[:, 128:256], func=AF.Exp, scale=-1.0
            )

            # mu tail on DVE: m2 = (nbMU + b2mu)*yd2 (in place over yd2),
            # v = ysq - m2 (in place over ysq),
            # fin = (v - Ey2)*iv accumulated per partition into acct col 0
            d_m2 = nc.vector.scalar_tensor_tensor(
                out=yd22, in0=nbMU, scalar=b2mu, in1=yd22,
                op0=OP.add, op1=OP.mult,
            )
            d_v = nc.vector.tensor_tensor(
                out=ysq2, in0=ysq2, in1=yd22, op=OP.subtract
            )
            d_fin_a = nc.vector.scalar_tensor_tensor(
                out=ivd[:, 0:128], in0=ysq2[:, 0:128], scalar=ey2c,
                in1=ivd[:, 0:128],
                op0=OP.subtract, op1=OP.mult, accum_out=acct[:, 0:1],
            )
            d_fin_b = nc.vector.scalar_tensor_tensor(
                out=ivd[:, 128:256], in0=ysq2[:, 128:256], scalar=ey2c,
                in1=ivd[:, 128:256],
                op0=OP.subtract, op1=OP.mult, accum_out=acct[:, 1:2],
            )

            out_ps = ps.tile([4, 1], f32, tag="outps")
            mm_acc = nc.tensor.matmul(out_ps, acct[:, 0:4], ones, start=True, stop=True)

            pe_order = [
                dlv0, dlv1, dmu0, dmu1,
                w2lv0, w2lv1, w2mu0, w2mu1, mm_acc,
            ]
            act_order = [r_lv0, r_lv1, r_mu0, a_tanh, a_exp_a, a_exp_b]
            dve_order = [r_mu1, d_m2, d_v, d_fin_a, d_fin_b]
            for order in (pe_order, act_order, dve_order):
                for a_i, b_i in zip(order[1:], order[:-1]):
                    add_dep_helper(a_i.ins, b_i.ins, sync=False, reason="stream-order")

            out_sb = sb.tile([4, 1], f32, tag="outsb")
            nc.vector.tensor_copy(out_sb, out_ps)
            nc.sync.dma_start(out=acc_d[:, :], in_=out_sb, single_packet=True)

    nc.compile()
    return nc


def pack_inputs(inputs: dict) -> list[dict]:
    import ml_dtypes

    bf = ml_dtypes.bfloat16
    f8 = ml_dtypes.float8_e4m3fn
    x = np.asarray(inputs["x_samples"], dtype=np.float32)
    y = np.ascontiguousarray(np.asarray(inputs["y_samples"], dtype=np.float32))
    mu_W1 = np.asarray(inputs["mu_W1"], dtype=np.float32)
    mu_b1 = np.asarray(inputs["mu_b1"], dtype=np.float32)
    mu_W2 = np.asarray(inputs["mu_W2"], dtype=np.float32)
    mu_b2 = np.asarray(inputs["mu_b2"], dtype=np.float32)
    lv_W1 = np.asarray(inputs["lv_W1"], dtype=np.float32)
    lv_b1 = np.asarray(inputs["lv_b1"], dtype=np.float32)
    lv_W2 = np.asarray(inputs["lv_W2"], dtype=np.float32)
    lv_b2 = np.asarray(inputs["lv_b2"], dtype=np.float32)

    def f32cols(a):  # [P] f32 -> [P, 2] bf16 raw-bit view
        return np.ascontiguousarray(a.astype(np.float32)[:, None]).view(bf)

    def drpack(wT8):  # [192, M] -> [96, 2M] fp8 bytes (DoubleRow groups)
        k, m = 96, wT8.shape[1]
        out = np.zeros((96, 2 * m), f8)
        out[:, 0:m] = wT8[0:96]
        out[:, m : 2 * m] = wT8[96:192]
        return out.view(np.uint8)

    # pa: byte-level build (fp8 payload), viewed as bf16 at the end
    pa_bytes_base = np.zeros((96, 1024), np.uint8)
    w1lv8 = (lv_W1.T * 8.0).astype(f8)  # [192, 128]
    w1mu8 = (mu_W1.T * 8.0).astype(f8)
    pa_bytes_base[0:96, 0:256] = drpack(w1lv8)
    pa_bytes_base[0:96, 256:512] = drpack(w1mu8)

    pb_base = np.zeros((128, WB), bf)
    pb_base[:, 0:64] = (lv_W2.T / 8.0).astype(bf)
    pb_base[:, 64:128] = (mu_W2.T / 8.0).astype(bf)
    pb_base[:, 640:642] = f32cols(np.broadcast_to(8.0 * lv_b1, (128,)))
    pb_base[:, 642:644] = f32cols(np.broadcast_to(8.0 * mu_b1, (128,)))
    pb_base[:, 644:646] = f32cols(np.tile(lv_b2, 2))
    pb_base[:, 646:648] = f32cols(np.tile(mu_b2, 2))
    pb_base[:, 650:652] = f32cols(np.ones(128, np.float32))

    x8 = x.astype(f8).view(np.uint8)  # [B, 192, 512]

    in_maps = []
    for bi in range(NCORES):
        pa_bytes = pa_bytes_base.copy()
        pa_bytes[0:96, 512:768] = x8[bi, 0:96, 0:256]
        pa_bytes[0:96, 768:1024] = x8[bi, 96:192, 0:256]
        pa = pa_bytes.view(bf)
        px_bytes = np.zeros((96, 512), np.uint8)
        px_bytes[0:96, 0:256] = x8[bi, 0:96, 256:512]
        px_bytes[0:96, 256:512] = x8[bi, 96:192, 256:512]
        px = px_bytes.view(bf)

        pb = pb_base.copy()
        yb = y[bi]  # [64, 512]
        ey = yb.mean(axis=1)
        ysq = yb * yb
        ey2 = ysq.mean(axis=1)
        yd2 = 2.0 * (yb - ey[:, None])
        pb[0:64, 128:384] = ysq[:, 0:256].astype(bf)
        pb[64:128, 128:384] = ysq[:, 256:512].astype(bf)
        pb[0:64, 384:640] = yd2[:, 0:256].astype(bf)
        pb[64:128, 384:640] = yd2[:, 256:512].astype(bf)
        pb[:, 648:650] = f32cols(np.tile(ey2, 2))

        in_maps.append({"pa": pa, "px": px, "pb": pb})
    return in_maps


def _combine(results) -> float:
    tot = 0.0
    for r in results:
        a = r["acc"].astype(np.float64)  # [4, 1]
        tot += a[0, 0] + a[1, 0]  # sum ((ysq - m2) - Ey2)*iv, two L-quarters
    return tot


def kernel(**inputs) -> np.ndarray:
    from concourse.bass_utils import run_bass_kernel_spmd

    if "nc" not in _CACHE:
        _CACHE["nc"] = build_nc(debug=False)
    nc = _CACHE["nc"]

    in_maps = pack_inputs(inputs)
    res = run_bass_kernel_spmd(nc, in_maps, core_ids=list(range(NCORES)))
    loss = -0.5 * _combine(res.results) / (B * L)
    return np.array(loss, dtype=np.float32)


# revision 32
# speedup vs baseline: 1.0457x; 1.0457x over previous
"""CLUB loss kernel for 8x TRN2 NeuronCores.

Math: per sample b (L=512 positions, D=64 dims):
  mu     = MLP_mu(x);  logvar = tanh(MLP_lv(x));  iv = exp(-logvar)
  loss = -0.5/(B*L) * sum_{b,d,l} [ ((ysq - Ey2) - mu*yd2) * iv ]
with ysq = y^2, yd2 = 2*(y - Ey); Ey/Ey2 per-(b,d) means over l.

y never feeds a matmul, so ysq/yd2/Ey2 are precomputed host-side and
shipped (bf16) instead of y.

Layer 1 runs in fp8 e4m3 DoubleRow mode (2 MACs/PE-row/cycle): the
192-channel contraction packs as 96 partitions x 2 rows, so one
matmul per (path, L-half) replaces the bf16 a/b split pair and the PE
spine halves. w1 ships x8 (lifting ~N(0,0.05) weights out of the e4m3
subnormal range); relu is positive-homogeneous so hs = relu(8h + 8*b1)
and the 8x cancels via w2/8 shipped host-side. Quantization errors are
random-sign across 32K summed terms; measured end-to-end error stays
~1e-3.

Everything after layer 1 runs in a (d, L-half) stacked layout -
partition p<64 is (d=p, half 0), p>=64 is (d=p-64, half 1) - so
tanh/exp/m2/v/fin are single full-width [128, 256] ops. Ey2 is folded
into the final DVE op: fin = ((ysq - m2) - Ey2)*iv accumulated
per-partition, so the scalar loss needs only one ones-vector collapse
matmul and no ACT accumulator on the critical path.

DMA: two packed [128, W] bf16-typed inputs (fp8/f32 regions ride in
them and are bitcast on-chip), triggered on SP and ACT HWDGE, 128
descriptors each. Output store is [4, 1] f32.

Sharding: data-parallel over batch B=8, one sample per core; host
does the tiny final combine.
"""

import sys

if "/opt/trn_rl_repo" not in sys.path:
    sys.path.insert(0, "/opt/trn_rl_repo")

import numpy as np

B, L = 8, 512
XD, YD, H = 192, 64, 128
NCORES = 8
HC = L // 2

WA = 768   # (pa+px) bf16-cols: w1lv8 128 | w1mu8 128 | x8 half0 | x8 half1
WB = 640   # w2lv 64 | w2mu 64 | ysq2 256 | yd22 256

_CACHE: dict = {}


def build_nc(debug: bool = False):
    import concourse.bass as bass
    import concourse.bacc as bacc
    import concourse.tile as tile
    from concourse import mybir
    from concourse.tile import add_dep_helper

    f32 = mybir.dt.float32
    bf16 = mybir.dt.bfloat16
    f8 = mybir.dt.float8e4
    AF = mybir.ActivationFunctionType
    OP = mybir.AluOpType
    DR = mybir.MatmulPerfMode.DoubleRow

    nc = bacc.Bacc("TRN2", target_bir_lowering=False, debug=debug)

    pa_d = nc.dram_tensor("pa", [96, 512], bf16, kind="ExternalInput")
    px_d = nc.dram_tensor("px", [96, 256], bf16, kind="ExternalInput")
    pb_d = nc.dram_tensor("pb", [128, WB], bf16, kind="ExternalInput")
    pc_d = nc.dram_tensor("pc", [128, 12], bf16, kind="ExternalInput")
    acc_d = nc.dram_tensor("acc", [4, 1], f32, kind="ExternalOutput")

    with tile.TileContext(nc) as tc:
        with (
            tc.tile_pool(name="sb", bufs=1) as sb,
            tc.tile_pool(name="ps", bufs=1, space=bass.MemorySpace.PSUM) as ps,
        ):
            pa = sb.tile([96, 512], bf16, tag="pa")
            mm_pa = nc.sync.dma_start(out=pa, in_=pa_d[:, :])
            pc = sb.tile([128, 12], bf16, tag="pc")
            mm_pc = nc.sync.dma_start(out=pc, in_=pc_d[:, :])
            add_dep_helper(mm_pc.ins, mm_pa.ins, sync=False, reason="sp-q-order")
            px = sb.tile([96, 256], bf16, tag="px")
            mm_px = nc.scalar.dma_start(out=px, in_=px_d[:, :])
            pb = sb.tile([128, WB], bf16, tag="pb")
            mm_pb = nc.scalar.dma_start(out=pb, in_=pb_d[:, :])
            add_dep_helper(mm_pb.ins, mm_px.ins, sync=False, reason="act-q-order")

            def dr3(ap, m):  # [96, 2m fp8] -> [96, 2, m] DoubleRow operand
                return ap.bitcast(f8).rearrange("p (two f) -> p two f", two=2)

            w1lv8 = dr3(pa[0:96, 0:128], 128)     # [96, 2, 128]
            w1mu8 = dr3(pa[0:96, 128:256], 128)
            x8 = [dr3(pa[0:96, 256:512], 256), dr3(px[0:96, 0:256], 256)]
            w2lvT = pb[:, 0:64]    # w2/8, bf16
            w2muT = pb[:, 64:128]
            ysq2 = pb[:, 128:384]      # (d, half) stacked
            yd22 = pb[:, 384:640]
            b1lv8 = pc[:, 0:2].bitcast(f32)   # 8*b1
            b1mu8 = pc[:, 2:4].bitcast(f32)
            b2lv = pc[:, 4:6].bitcast(f32)    # rows duplicated per half
            b2mu = pc[:, 6:8].bitcast(f32)
            ey2c = pc[:, 8:10].bitcast(f32)   # Ey2 dup
            ones = pc[:, 10:12].bitcast(f32)

            acct = sb.tile([128, 4], f32, tag="acct")
            nc.gpsimd.memset(acct, 0.0)

            hs_lv = sb.tile([128, L], bf16, tag="hslv")
            hs_mu = sb.tile([128, L], bf16, tag="hsmu")
            tt = sb.tile([128, HC], f32, tag="tt")
            ivd = sb.tile([128, HC], bf16, tag="ivd")

            # layer 1: fp8 DoubleRow, one matmul per (path, half)
            h_lv0 = ps.tile([128, HC], f32, tag="hlv0")
            h_lv1 = ps.tile([128, HC], f32, tag="hlv1")
            h_mu0 = ps.tile([128, HC], f32, tag="hmu0")
            h_mu1 = ps.tile([128, HC], f32, tag="hmu1")
            dlv0 = nc.tensor.matmul(h_lv0, w1lv8, x8[0], start=True, stop=True,
                                    perf_mode=DR)
            dlv1 = nc.tensor.matmul(h_lv1, w1lv8, x8[1], start=True, stop=True,
                                    perf_mode=DR)
            dmu0 = nc.tensor.matmul(h_mu0, w1mu8, x8[0], start=True, stop=True,
                                    perf_mode=DR)
            dmu1 = nc.tensor.matmul(h_mu1, w1mu8, x8[1], start=True, stop=True,
                                    perf_mode=DR)

            # relus emit 8*relu(h + b1); the 8x cancels in w2/8.
            # relu_mu1 runs on DVE to keep the ACT spine short.
            r_lv0 = nc.scalar.activation(
                out=hs_lv[:, 0:HC], in_=h_lv0, func=AF.Relu, bias=b1lv8, scale=1.0
            )
            r_lv1 = nc.scalar.activation(
                out=hs_lv[:, HC:L], in_=h_lv1, func=AF.Relu, bias=b1lv8, scale=1.0
            )
            r_mu0 = nc.scalar.activation(
                out=hs_mu[:, 0:HC], in_=h_mu0, func=AF.Relu, bias=b1mu8, scale=1.0
            )
            r_mu1 = nc.vector.tensor_scalar(
                out=hs_mu[:, HC:L], in0=h_mu1, scalar1=b1mu8, scalar2=0.0,
                op0=OP.add, op1=OP.max,
            )

            # layer 2 (bf16) into (d, half) stacked PSUM tiles
            nbLV = ps.tile([128, HC], f32, tag="nblv")
            nbMU = ps.tile([128, HC], f32, tag="nbmu")
            w2lv0 = nc.tensor.matmul(
                nbLV[0:64, :], w2lvT, hs_lv[:, 0:HC], start=True, stop=True
            )
            w2lv1 = nc.tensor.matmul(
                nbLV[64:128, :], w2lvT, hs_lv[:, HC:L], start=True, stop=True
            )
            w2mu0 = nc.tensor.matmul(
                nbMU[0:64, :], w2muT, hs_mu[:, 0:HC], start=True, stop=True
            )
            w2mu1 = nc.tensor.matmul(
                nbMU[64:128, :], w2muT, hs_mu[:, HC:L], start=True, stop=True
            )

            # lv tail: tanh(+b2lv) -> exp(-.)
            a_tanh = nc.scalar.activation(
                out=tt, in_=nbLV, func=AF.Tanh, bias=b2lv, scale=1.0
            )
            a_exp_a = nc.scalar.activation(
                out=ivd[:, 0:128], in_=tt[:, 0:128], func=AF.Exp, scale=-1.0
            )
            a_exp_b = nc.scalar.activation(
                out=ivd[:, 128:256], in_=tt[:, 128:256], func=AF.Exp, scale=-1.0
            )

            # mu tail on DVE: m2 = (nbMU + b2mu)*yd2 (in place over yd2),
            # v = ysq - m2 (in place over ysq),
            # fin = (v - Ey2)*iv accumulated per partition into acct col 0
            d_m2 = nc.vector.scalar_tensor_tensor(
                out=yd22, in0=nbMU, scalar=b2mu, in1=yd22,
                op0=OP.add, op1=OP.mult,
            )
            d_v = nc.vector.tensor_tensor(
                out=ysq2, in0=ysq2, in1=yd22, op=OP.subtract
            )
            d_fin_a = nc.vector.scalar_tensor_tensor(
                out=ivd[:, 0:128], in0=ysq2[:, 0:128], scalar=ey2c,
                in1=ivd[:, 0:128],
                op0=OP.subtract, op1=OP.mult, accum_out=acct[:, 0:1],
            )
            d_fin_b = nc.vector.scalar_tensor_tensor(
                out=ivd[:, 128:256], in0=ysq2[:, 128:256], scalar=ey2c,
                in1=ivd[:, 128:256],
                op0=OP.subtract, op1=OP.mult, accum_out=acct[:, 1:2],
            )

            out_ps = ps.tile([4, 1], f32, tag="outps")
            mm_acc = nc.tensor.matmul(out_ps, acct[:, 0:4], ones, start=True, stop=True)

            pe_order = [
                dlv0, dlv1, dmu0, dmu1,
                w2lv0, w2lv1, w2mu0, w2mu1, mm_acc,
            ]
            act_order = [r_lv0, r_lv1, r_mu0, a_tanh, a_exp_a, a_exp_b]
            dve_order = [r_mu1, d_m2, d_v, d_fin_a, d_fin_b]
            for order in (pe_order, act_order, dve_order):
                for a_i, b_i in zip(order[1:], order[:-1]):
                    add_dep_helper(a_i.ins, b_i.ins, sync=False, reason="stream-order")

            out_sb = sb.tile([4, 1], f32, tag="outsb")
            nc.vector.tensor_copy(out_sb, out_ps)
            nc.sync.dma_start(out=acc_d[:, :], in_=out_sb, single_packet=True)

    nc.compile()
    return nc


def pack_inputs(inputs: dict) -> list[dict]:
    import ml_dtypes

    bf = ml_dtypes.bfloat16
    f8 = ml_dtypes.float8_e4m3fn
    x = np.asarray(inputs["x_samples"], dtype=np.float32)
    y = np.ascontiguousarray(np.asarray(inputs["y_samples"], dtype=np.float32))
    mu_W1 = np.asarray(inputs["mu_W1"], dtype=np.float32)
    mu_b1 = np.asarray(inputs["mu_b1"], dtype=np.float32)
    mu_W2 = np.asarray(inputs["mu_W2"], dtype=np.float32)
    mu_b2 = np.asarray(inputs["mu_b2"], dtype=np.float32)
    lv_W1 = np.asarray(inputs["lv_W1"], dtype=np.float32)
    lv_b1 = np.asarray(inputs["lv_b1"], dtype=np.float32)
    lv_W2 = np.asarray(inputs["lv_W2"], dtype=np.float32)
    lv_b2 = np.asarray(inputs["lv_b2"], dtype=np.float32)

    def f32cols(a):  # [P] f32 -> [P, 2] bf16 raw-bit view
        return np.ascontiguousarray(a.astype(np.float32)[:, None]).view(bf)

    def drpack(wT8):  # [192, M] -> [96, 2M] fp8 bytes (DoubleRow groups)
        k, m = 96, wT8.shape[1]
        out = np.zeros((96, 2 * m), f8)
        out[:, 0:m] = wT8[0:96]
        out[:, m : 2 * m] = wT8[96:192]
        return out.view(np.uint8)

    # pa: byte-level build (fp8 payload), viewed as bf16 at the end
    pa_bytes_base = np.zeros((96, 1024), np.uint8)
    w1lv8 = (lv_W1.T * 8.0).astype(f8)  # [192, 128]
    w1mu8 = (mu_W1.T * 8.0).astype(f8)
    pa_bytes_base[0:96, 0:256] = drpack(w1lv8)
    pa_bytes_base[0:96, 256:512] = drpack(w1mu8)

    pb_base = np.zeros((128, WB), bf)
    pb_base[:, 0:64] = (lv_W2.T / 8.0).astype(bf)
    pb_base[:, 64:128] = (mu_W2.T / 8.0).astype(bf)

    pc_base = np.zeros((128, 12), bf)
    pc_base[:, 0:2] = f32cols(np.broadcast_to(8.0 * lv_b1, (128,)))
    pc_base[:, 2:4] = f32cols(np.broadcast_to(8.0 * mu_b1, (128,)))
    pc_base[:, 4:6] = f32cols(np.tile(lv_b2, 2))
    pc_base[:, 6:8] = f32cols(np.tile(mu_b2, 2))
    pc_base[:, 10:12] = f32cols(np.ones(128, np.float32))

    x8 = x.astype(f8).view(np.uint8)  # [B, 192, 512]

    in_maps = []
    for bi in range(NCORES):
        pa_bytes = pa_bytes_base.copy()
        pa_bytes[0:96, 512:768] = x8[bi, 0:96, 0:256]
        pa_bytes[0:96, 768:1024] = x8[bi, 96:192, 0:256]
        pa = pa_bytes.view(bf)
        px_bytes = np.zeros((96, 512), np.uint8)
        px_bytes[0:96, 0:256] = x8[bi, 0:96, 256:512]
        px_bytes[0:96, 256:512] = x8[bi, 96:192, 256:512]
        px = px_bytes.view(bf)

        pb = pb_base.copy()
        yb = y[bi]  # [64, 512]
        ey = yb.mean(axis=1)
        ysq = yb * yb
        ey2 = ysq.mean(axis=1)
        yd2 = 2.0 * (yb - ey[:, None])
        pb[0:64, 128:384] = ysq[:, 0:256].astype(bf)
        pb[64:128, 128:384] = ysq[:, 256:512].astype(bf)
        pb[0:64, 384:640] = yd2[:, 0:256].astype(bf)
        pb[64:128, 384:640] = yd2[:, 256:512].astype(bf)
        pc = pc_base.copy()
        pc[:, 8:10] = f32cols(np.tile(ey2, 2))

        in_maps.append({"pa": pa, "px": px, "pb": pb, "pc": pc})
    return in_maps


def _combine(results) -> float:
    tot = 0.0
    for r in results:
        a = r["acc"].astype(np.float64)  # [4, 1]
        tot += a[0, 0] + a[1, 0]  # sum ((ysq - m2) - Ey2)*iv, two L-quarters
    return tot


def kernel(**inputs) -> np.ndarray:
    from concourse.bass_utils import run_bass_kernel_spmd

    if "nc" not in _CACHE:
        _CACHE["nc"] = build_nc(debug=False)
    nc = _CACHE["nc"]

    in_maps = pack_inputs(inputs)
    res = run_bass_kernel_spmd(nc, in_maps, core_ids=list(range(NCORES)))
    loss = -0.5 * _combine(res.results) / (B * L)
    return np.array(loss, dtype=np.float32)
